# revision 1
# baseline (speedup 1.0000x reference)
"""MoE gate (router) kernel for Trainium2.

Computes, for hidden_states [T, H] and gate weight [E, H]:
    logits = hidden_states @ weight.T          # [T, E]
    probs  = softmax(logits, axis=-1)
    topk_weight, topk_idx = top_k(probs, 8)    # normalized over the top-8
    row_idx = arange(T*8).reshape(8, T).T

Strategy (8 NeuronCores, data parallel over tokens):
  - Host pre-transposes: each core receives hsT [H, T/8] and wT [H, E] so the
    contraction dim H lands on SBUF partitions with fully-contiguous DMA --
    no on-device transposes at all.
  - fp32 accuracy from fp16 hi/lo splits (host-side, same DMA bytes as f32):
    hs = hi + lo/2^11, 64*w = whi + wlo/2^11, with each part fp16 (11-bit
    mantissa, so ~22 mantissa bits total; the dropped lo*lo term is ~2^-22).
    The scaling keeps the lo parts in fp16 normal range.  Native fp32 matmul
    would be 4 cycles/row and trips a walrus codegen limit on sync waits for
    self-loading fp32 LDWEIGHTS; fp16 runs 1 cycle/row.
  - Per k-tile only TWO matmuls: rhs = [whi | wlo] concatenated [128 x 512]
    shares one weight load for the hi*hi and hi*lo terms; the lo*hi term
    accumulates into the same scaled-2^11 PSUM columns as hi*lo:
        psum[:, 0:256]   += hshi . whi
        psum[:, 256:512] += hshi . wlo + hslo . whi
    logits = 2^-6 * psum[:, 0:256] + 2^-17 * psum[:, 256:512]
  - DVE max/max_index give the top-8 values + indices per token in one
    instruction each.  Softmax over the full 256 experts followed by top-k
    renormalization reduces algebraically to a softmax over just the top-8
    logits, so the full-row softmax is never materialized.
"""

import numpy as np

TOP_K = 8
NUM_EXPERTS = 256
HIDDEN = 7168
NUM_TOKENS = 16384
N_CORES = 8
T_LOC = NUM_TOKENS // N_CORES

W_SCALE = 64.0       # weight pre-scale so fp16(64*w) stays normal-range
LO_SCALE = 2048.0    # 2^11: lo parts carry the next 11 mantissa bits

_NC_CACHE = {}


def build_gate_nc(t_loc=T_LOC, h=HIDDEN, e=NUM_EXPERTS, repeat=1):
    import concourse.mybir as mybir
    import concourse.tile as tile
    from concourse import bacc

    f32 = mybir.dt.float32
    fp16 = mybir.dt.float16
    P = 128
    KT = h // P          # k-tiles along hidden dim
    TS = t_loc // P      # 128-token subtiles per core
    KC = 8 if KT % 8 == 0 else (4 if KT % 4 == 0 else 1)  # k-tiles per DMA
    NKC = KT // KC       # number of k-chunks

    nc = bacc.Bacc("TRN2", target_bir_lowering=False)
    hsT_hi = nc.dram_tensor("hsT_hi", [h, t_loc], fp16, kind="ExternalInput")
    hsT_lo = nc.dram_tensor("hsT_lo", [h, t_loc], fp16, kind="ExternalInput")
    # wT_cat[:, 0:e] = fp16(64*wT), wT_cat[:, e:2e] = fp16((64*wT - hi) * 2^11)
    wT_cat = nc.dram_tensor("wT_cat", [h, 2 * e], fp16, kind="ExternalInput")
    idx_out = nc.dram_tensor(
        "topk_idx", [t_loc, TOP_K], mybir.dt.int32, kind="ExternalOutput"
    )
    w_out = nc.dram_tensor("topk_w", [t_loc, TOP_K], f32, kind="ExternalOutput")

    # [128, KT, *] views with H split over partitions
    hshi_t = hsT_hi[:, :].rearrange("(ko p) t -> p ko t", p=P)
    hslo_t = hsT_lo[:, :].rearrange("(ko p) t -> p ko t", p=P)
    wcat_t = wT_cat[:, :].rearrange("(ko p) e -> p ko e", p=P)

    with tile.TileContext(nc) as tc:
        with (
            tc.tile_pool(name="wpool", bufs=1) as wpool,
            tc.tile_pool(name="hpool", bufs=28) as hpool,
            tc.tile_pool(name="lpool", bufs=3) as lpool,
            tc.tile_pool(name="spool", bufs=4) as spool,
            tc.tile_pool(name="psum", bufs=4, space="PSUM") as psum_pool,
        ):
            # output staging: small per-tile results accumulate here and leave
            # as two large descriptor DMAs at the end (tiny per-tile DMAs get
            # the DIRECT2D encoding whose single wait slot walrus overflows)
            stage_idx = wpool.tile([P, TS, TOP_K], mybir.dt.int32, tag="sidx")
            stage_wv = wpool.tile([P, TS, TOP_K], f32, tag="swv")
            # gate weight: resident in SBUF, one tile per k-chunk so each
            # matmul depends on exactly one weight-load DMA
            wt_chunks = []
            for kc in range(NKC):
                wc = wpool.tile([P, KC, 2 * e], fp16, tag=f"wt{kc}", name=f"wt{kc}")
                nc.sync.dma_start(wc, wcat_t[:, kc * KC : (kc + 1) * KC, :])
                wt_chunks.append(wc)
            for rep in range(repeat):
                for ts_i in range(TS):
                    tslc = slice(ts_i * P, (ts_i + 1) * P)
                    hs_chunks = []
                    for kc in range(NKC):
                        kslc = slice(kc * KC, (kc + 1) * KC)
                        hhi = hpool.tile(
                            [P, KC, P], fp16, tag="hs", name=f"hshi{rep}_{ts_i}_{kc}"
                        )
                        nc.sync.dma_start(hhi, hshi_t[:, kslc, tslc])
                        hlo = hpool.tile(
                            [P, KC, P], fp16, tag="hs", name=f"hslo{rep}_{ts_i}_{kc}"
                        )
                        nc.sync.dma_start(hlo, hslo_t[:, kslc, tslc])
                        hs_chunks.append((hhi, hlo))
                    pt = psum_pool.tile([P, 2 * e], f32, tag="pt")
                    for k in range(KT):
                        kc, ki = divmod(k, KC)
                        hhi, hlo = hs_chunks[kc]
                        wc = wt_chunks[kc]
                        # psum[:, 0:2e] += hshi . [whi | wlo]
                        nc.tensor.matmul(
                            pt,
                            hhi[:, ki, :],
                            wc[:, ki, :],
                            start=(k == 0),
                            stop=False,
                        )
                        # psum[:, e:2e] += hslo . whi   (same 2^11 scale as hi*lo)
                        nc.tensor.matmul(
                            pt[:, e:],
                            hlo[:, ki, :],
                            wc[:, ki, :e],
                            start=False,
                            stop=(k == KT - 1),
                        )
                    # logits = 2^-6 * psum_hi + 2^-17 * psum_cross
                    cross = lpool.tile([P, e], f32, tag="cross")
                    nc.vector.tensor_scalar_mul(cross, pt[:, e:], 1.0 / (64.0 * 2048.0))
                    logits = lpool.tile([P, e], f32, tag="logits")
                    nc.vector.tensor_scalar(
                        logits,
                        pt[:, :e],
                        1.0 / 64.0,
                        None,
                        mybir.AluOpType.mult,
                    )
                    nc.vector.tensor_add(logits, logits, cross)
                    mx = spool.tile([P, TOP_K], f32, tag="mx")
                    nc.vector.max(out=mx, in_=logits)
                    idx_u = spool.tile([P, TOP_K], mybir.dt.uint32, tag="idxu")
                    nc.vector.max_index(idx_u, mx, logits)
                    nc.vector.tensor_copy(stage_idx[:, ts_i, :], idx_u)
                    # normalized top-k softmax: exp(v - v_max) / sum
                    nm = spool.tile([P, 1], f32, tag="nm")
                    nc.vector.tensor_scalar_mul(nm, mx[:, 0:1], -1.0)
                    ev = spool.tile([P, TOP_K], f32, tag="ev")
                    sm = spool.tile([P, 1], f32, tag="sm")
                    nc.scalar.activation(
                        ev,
                        mx,
                        mybir.ActivationFunctionType.Exp,
                        bias=nm,
                        scale=1.0,
                        accum_out=sm,
                    )
                    rc = spool.tile([P, 1], f32, tag="rc")
                    nc.vector.reciprocal(rc, sm)
                    nc.vector.tensor_scalar_mul(stage_wv[:, ts_i, :], ev, rc)
            nc.sync.dma_start(
                idx_out[:, :].rearrange("(ts p) k -> p ts k", p=P), stage_idx
            )
            nc.sync.dma_start(
                w_out[:, :].rearrange("(ts p) k -> p ts k", p=P), stage_wv
            )
    nc.compile()
    return nc


def _get_nc():
    key = (T_LOC, HIDDEN, NUM_EXPERTS)
    if key not in _NC_CACHE:
        _NC_CACHE[key] = build_gate_nc(*key)
    return _NC_CACHE[key]


def _split_fp16(x, pre_scale=1.0):
    """x (f32) -> (hi, lo) fp16 with hi + lo/2^11 ~= pre_scale*x."""
    xs = x * np.float32(pre_scale) if pre_scale != 1.0 else x
    hi = xs.astype(np.float16)
    lo = ((xs - hi.astype(np.float32)) * np.float32(LO_SCALE)).astype(np.float16)
    return hi, lo


def _prep_inputs(hs, w):
    wT = np.ascontiguousarray(w.T)  # [H, E]
    w_hi, w_lo = _split_fp16(wT, W_SCALE)
    wT_cat = np.concatenate([w_hi, w_lo], axis=1)  # [H, 2E]
    in_maps = []
    for c in range(N_CORES):
        hsT_c = np.ascontiguousarray(hs[c * T_LOC : (c + 1) * T_LOC].T)  # [H, T_LOC]
        hs_hi, hs_lo = _split_fp16(hsT_c)
        in_maps.append({"hsT_hi": hs_hi, "hsT_lo": hs_lo, "wT_cat": wT_cat})
    return in_maps


_FN_CACHE = {}


def _make_runner(nc):
    """Compile a reusable 8-core PJRT callable (same lowering path as
    run_bass_kernel_spmd under axon, but cached so repeat kernel() calls
    skip re-tracing/compiling)."""
    import jax
    import concourse.mybir as mybir
    from concourse import bass2jax
    from jax.sharding import Mesh, NamedSharding, PartitionSpec
    from jax.experimental.shard_map import shard_map

    bass2jax.install_neuronx_cc_hook()
    partition_name = nc.partition_id_tensor.name if nc.partition_id_tensor else None
    in_names, out_names, out_avals, zero_shapes = [], [], [], []
    for alloc in nc.m.functions[0].allocations:
        if not isinstance(alloc, mybir.MemoryLocationSet):
            continue
        name = alloc.memorylocations[0].name
        if alloc.kind == "ExternalInput":
            if name != partition_name:
                in_names.append(name)
        elif alloc.kind == "ExternalOutput":
            shape = tuple(alloc.tensor_shape)
            dtype = mybir.dt.np(alloc.dtype)
            out_names.append(name)
            out_avals.append(jax.core.ShapedArray(shape, dtype))
            zero_shapes.append((shape, dtype))
    n_params = len(in_names)
    n_outs = len(out_avals)
    all_in_names = list(in_names) + list(out_names)
    if partition_name is not None:
        all_in_names.append(partition_name)

    def _body(*args):
        operands = list(args)
        if partition_name is not None:
            operands.append(bass2jax.partition_id_tensor())
        outs = bass2jax._bass_exec_p.bind(
            *operands,
            out_avals=tuple(out_avals),
            in_names=tuple(all_in_names),
            out_names=tuple(out_names),
            lowering_input_output_aliases=(),
            sim_require_finite=True,
            sim_require_nnan=True,
            nc=nc,
        )
        return tuple(outs)

    devices = jax.devices()[:N_CORES]
    mesh = Mesh(np.asarray(devices), ("core",))
    in_specs = (PartitionSpec("core"),) * (n_params + n_outs)
    out_specs = (PartitionSpec("core"),) * len(out_names)
    donate = tuple(range(n_params, n_params + n_outs))
    fn = jax.jit(
        shard_map(
            _body, mesh=mesh, in_specs=in_specs, out_specs=out_specs, check_rep=False
        ),
        donate_argnums=donate,
        keep_unused=True,
    )
    sharding = NamedSharding(mesh, PartitionSpec("core"))

    def run(in_maps):
        concat_in = [
            np.concatenate(
                [np.asarray(in_maps[c][nm]) for c in range(N_CORES)], axis=0
            )
            for nm in in_names
        ]
        zeros = [
            np.zeros((N_CORES * s[0], *s[1:]), dt) for s, dt in zero_shapes
        ]
        dev_in = [jax.device_put(x, sharding) for x in concat_in]
        out_arrs = fn(*dev_in, *zeros)
        return [
            {
                nm: np.asarray(out_arrs[i]).reshape(
                    N_CORES, *out_avals[i].shape
                )[c]
                for i, nm in enumerate(out_names)
            }
            for c in range(N_CORES)
        ]

    return run


def kernel(hidden_states, weight):
    hs = np.asarray(hidden_states, dtype=np.float32)
    w = np.asarray(weight, dtype=np.float32)
    assert hs.shape == (NUM_TOKENS, HIDDEN), hs.shape
    assert w.shape == (NUM_EXPERTS, HIDDEN), w.shape

    in_maps = _prep_inputs(hs, w)
    nc = _get_nc()
    try:
        if "run" not in _FN_CACHE:
            _FN_CACHE["run"] = _make_runner(nc)
        results = _FN_CACHE["run"](in_maps)
    except Exception:
        # fall back to the stock path if the cached-runner path breaks
        from concourse.bass_utils import run_bass_kernel_spmd

        results = run_bass_kernel_spmd(
            nc, in_maps, core_ids=list(range(N_CORES))
        ).results

    topk_idx = np.concatenate([r["topk_idx"] for r in results], axis=0)
    topk_w = np.concatenate([r["topk_w"] for r in results], axis=0)
    row_idx = (
        np.arange(NUM_TOKENS * TOP_K, dtype=np.int32).reshape(TOP_K, NUM_TOKENS).T
    )
    return (
        topk_idx.astype(np.int32),
        topk_w.astype(np.float32),
        row_idx,
    )



# revision 2
# speedup vs baseline: 165.5322x; 165.5322x over previous
"""MoE gate (router) kernel for Trainium2.

Computes, for hidden_states [T, H] and gate weight [E, H]:
    logits = hidden_states @ weight.T          # [T, E]
    probs  = softmax(logits, axis=-1)
    topk_weight, topk_idx = top_k(probs, 8)    # normalized over the top-8
    row_idx = arange(T*8).reshape(8, T).T

Strategy (8 NeuronCores, data parallel over tokens):
  - fp32 accuracy from fp16 hi/lo splits (host-side, same DMA bytes as f32):
    hs = hi + lo/2^11, 64*w = whi + wlo/2^11, with each part fp16 (11-bit
    mantissa, ~22 mantissa bits total; the dropped lo*lo term is ~2^-22).
    fp16 matmul runs 1 cycle/row vs 4 for fp32, so 3 fp16 terms beat fp32.
  - Per k-tile TWO matmuls: rhs = [whi | wlo] concatenated [128 x 512]
    shares one weight load for the hi*hi and hi*lo terms; the lo*hi term
    accumulates into the same scaled-2^11 PSUM columns as hi*lo:
        psum[:, 0:256]   += hshi . whi
        psum[:, 256:512] += hshi . wlo + hslo . whi
    logits*64 = psum[:, 0:256] + 2^-11 * psum[:, 256:512]
  - Host prepacks hs into [128, TS, 2, KT, 128] (partition-major, hi/lo
    interleaved per token-tile) so each 128-token tile loads with ONE DMA
    whose per-partition run is 28,672 contiguous bytes (128 descriptors),
    instead of 14 strided DMAs x 1024 256-byte descriptors.  Weight is one
    resident DMA with 57,344-byte runs.  Descriptor count per iteration
    drops ~100x; DMA runs at full bus width.
  - Top-8 via DVE max/max_index on the raw 64x-scaled logits (scale
    invariant); the 1/64 folds into the exp activation's scale.  Softmax
    over 256 + top-k renorm reduces to softmax over the top-8 logits.
"""

import numpy as np

TOP_K = 8
NUM_EXPERTS = 256
HIDDEN = 7168
NUM_TOKENS = 16384
N_CORES = 8
T_LOC = NUM_TOKENS // N_CORES

W_SCALE = 64.0       # weight pre-scale so fp16(64*w) stays normal-range
LO_SCALE = 2048.0    # 2^11: lo parts carry the next 11 mantissa bits

P = 128
KT = HIDDEN // P     # 56 k-tiles along hidden dim
TS = T_LOC // P      # 16 128-token subtiles per core

_NC_CACHE = {}


def build_gate_nc(t_loc=T_LOC, h=HIDDEN, e=NUM_EXPERTS, repeat=1):
    import concourse.mybir as mybir
    import concourse.tile as tile
    from concourse import bacc

    f32 = mybir.dt.float32
    fp16 = mybir.dt.float16
    kt = h // P
    ts_n = t_loc // P

    nc = bacc.Bacc("TRN2", target_bir_lowering=False)
    # hs_pack[p, ts, c, ko, t]: c=0 hi part, c=1 lo part; hidden = ko*128+p,
    # token = ts*128+t.  Per-partition contiguous run = 2*kt*128*2 bytes.
    hs_pack = nc.dram_tensor(
        "hs_pack", [P, ts_n * 2 * kt * P], fp16, kind="ExternalInput"
    )
    # w_pack[p, ko, 0:e]=fp16(64*wT), [p, ko, e:2e]=fp16((64*wT - hi)*2^11)
    w_pack = nc.dram_tensor("w_pack", [P, kt * 2 * e], fp16, kind="ExternalInput")
    idx_out = nc.dram_tensor(
        "topk_idx", [t_loc, TOP_K], mybir.dt.int32, kind="ExternalOutput"
    )
    w_out = nc.dram_tensor("topk_w", [t_loc, TOP_K], f32, kind="ExternalOutput")

    hs_v = hs_pack[:, :].rearrange(
        "p (ts c ko t) -> p ts c ko t", ts=ts_n, c=2, ko=kt
    )
    w_v = w_pack[:, :].rearrange("p (ko e) -> p ko e", ko=kt)

    with tile.TileContext(nc) as tc:
        with (
            tc.tile_pool(name="wpool", bufs=1) as wpool,
            tc.tile_pool(name="hpool", bufs=3) as hpool,
            tc.tile_pool(name="lpool", bufs=3) as lpool,
            tc.tile_pool(name="spool", bufs=4) as spool,
            tc.tile_pool(name="psum", bufs=4, space="PSUM") as psum_pool,
        ):
            # output staging: small per-tile results accumulate here and leave
            # as two large descriptor DMAs at the end (tiny per-tile DMAs get
            # the DIRECT2D encoding whose single wait slot walrus overflows)
            stage_idx = wpool.tile([P, ts_n, TOP_K], mybir.dt.int32, tag="sidx")
            stage_wv = wpool.tile([P, ts_n, TOP_K], f32, tag="swv")
            # gate weight: resident in SBUF for the whole kernel
            wt = wpool.tile([P, kt, 2 * e], fp16, tag="wt", name="wt")
            nc.sync.dma_start(wt, w_v)
            for rep in range(repeat):
                for ts_i in range(ts_n):
                    ht = hpool.tile(
                        [P, 2, kt, P], fp16, tag="hs", name=f"hs{rep}_{ts_i}"
                    )
                    nc.sync.dma_start(ht, hs_v[:, ts_i])
                    pt = psum_pool.tile([P, 2 * e], f32, tag="pt")
                    for k in range(kt):
                        # psum[:, 0:2e] += hshi . [whi | wlo]
                        nc.tensor.matmul(
                            pt,
                            ht[:, 0, k, :],
                            wt[:, k, :],
                            start=(k == 0),
                            stop=False,
                        )
                        # psum[:, e:2e] += hslo . whi  (same 2^11 scale as hi*lo)
                        nc.tensor.matmul(
                            pt[:, e:],
                            ht[:, 1, k, :],
                            wt[:, k, :e],
                            start=False,
                            stop=(k == kt - 1),
                        )
                    # 64*logits = psum_hi + 2^-11 * psum_cross  (order-preserving)
                    cross = lpool.tile([P, e], f32, tag="cross")
                    nc.vector.tensor_scalar_mul(cross, pt[:, e:], 1.0 / LO_SCALE)
                    m = lpool.tile([P, e], f32, tag="m")
                    nc.vector.tensor_add(m, pt[:, :e], cross)
                    mx = spool.tile([P, TOP_K], f32, tag="mx")
                    nc.vector.max(out=mx, in_=m)
                    idx_u = spool.tile([P, TOP_K], mybir.dt.uint32, tag="idxu")
                    nc.vector.max_index(idx_u, mx, m)
                    nc.vector.tensor_copy(stage_idx[:, ts_i, :], idx_u)
                    # normalized top-k softmax on true logits = raw/64:
                    # exp((raw - raw_max)/64) / sum
                    nm = spool.tile([P, 1], f32, tag="nm")
                    nc.vector.tensor_scalar_mul(nm, mx[:, 0:1], -1.0 / W_SCALE)
                    ev = spool.tile([P, TOP_K], f32, tag="ev")
                    sm = spool.tile([P, 1], f32, tag="sm")
                    nc.scalar.activation(
                        ev,
                        mx,
                        mybir.ActivationFunctionType.Exp,
                        bias=nm,
                        scale=1.0 / W_SCALE,
                        accum_out=sm,
                    )
                    rc = spool.tile([P, 1], f32, tag="rc")
                    nc.vector.reciprocal(rc, sm)
                    nc.vector.tensor_scalar_mul(stage_wv[:, ts_i, :], ev, rc)
            nc.sync.dma_start(
                idx_out[:, :].rearrange("(ts p) k -> p ts k", p=P), stage_idx
            )
            nc.sync.dma_start(
                w_out[:, :].rearrange("(ts p) k -> p ts k", p=P), stage_wv
            )
    nc.compile()
    return nc


def _get_nc():
    key = (T_LOC, HIDDEN, NUM_EXPERTS)
    if key not in _NC_CACHE:
        _NC_CACHE[key] = build_gate_nc(*key)
    return _NC_CACHE[key]


def _split_fp16(x, pre_scale=1.0):
    """x (f32) -> (hi, lo) fp16 with hi + lo/2^11 ~= pre_scale*x."""
    xs = x * np.float32(pre_scale) if pre_scale != 1.0 else x
    hi = xs.astype(np.float16)
    lo = ((xs - hi.astype(np.float32)) * np.float32(LO_SCALE)).astype(np.float16)
    return hi, lo


def _prep_inputs(hs, w):
    wT = np.ascontiguousarray(w.T)  # [H, E]
    w_hi, w_lo = _split_fp16(wT, W_SCALE)
    wcat = np.concatenate([w_hi, w_lo], axis=1)  # [H, 2E]
    # [H, 2E] -> [p, ko, 2E] -> flat [P, KT*2E]
    w_pack = np.ascontiguousarray(
        wcat.reshape(KT, P, 2 * NUM_EXPERTS).transpose(1, 0, 2)
    ).reshape(P, KT * 2 * NUM_EXPERTS)
    in_maps = []
    for c in range(N_CORES):
        hsT_c = np.ascontiguousarray(hs[c * T_LOC : (c + 1) * T_LOC].T)  # [H,Tl]
        hs_hi, hs_lo = _split_fp16(hsT_c)
        # [H, Tl] = [ko*P, ts*P] -> [p, ts, ko, t]
        hi5 = hs_hi.reshape(KT, P, TS, P).transpose(1, 2, 0, 3)
        lo5 = hs_lo.reshape(KT, P, TS, P).transpose(1, 2, 0, 3)
        hs_pack = np.ascontiguousarray(
            np.stack([hi5, lo5], axis=2)  # [p, ts, 2, ko, t]
        ).reshape(P, TS * 2 * KT * P)
        in_maps.append({"hs_pack": hs_pack, "w_pack": w_pack})
    return in_maps


_FN_CACHE = {}


def _build_jit(nc, donate=True):
    """Build the reusable 8-core PJRT callable (same lowering path as
    run_bass_kernel_spmd under axon, but cached so repeat kernel() calls
    skip re-tracing/compiling)."""
    import jax
    import concourse.mybir as mybir
    from concourse import bass2jax
    from jax.sharding import Mesh, NamedSharding, PartitionSpec
    from jax.experimental.shard_map import shard_map

    bass2jax.install_neuronx_cc_hook()
    partition_name = nc.partition_id_tensor.name if nc.partition_id_tensor else None
    in_names, out_names, out_avals, zero_shapes = [], [], [], []
    for alloc in nc.m.functions[0].allocations:
        if not isinstance(alloc, mybir.MemoryLocationSet):
            continue
        name = alloc.memorylocations[0].name
        if alloc.kind == "ExternalInput":
            if name != partition_name:
                in_names.append(name)
        elif alloc.kind == "ExternalOutput":
            shape = tuple(alloc.tensor_shape)
            dtype = mybir.dt.np(alloc.dtype)
            out_names.append(name)
            out_avals.append(jax.core.ShapedArray(shape, dtype))
            zero_shapes.append((shape, dtype))
    n_params = len(in_names)
    n_outs = len(out_avals)
    all_in_names = list(in_names) + list(out_names)
    if partition_name is not None:
        all_in_names.append(partition_name)

    def _body(*args):
        operands = list(args)
        if partition_name is not None:
            operands.append(bass2jax.partition_id_tensor())
        outs = bass2jax._bass_exec_p.bind(
            *operands,
            out_avals=tuple(out_avals),
            in_names=tuple(all_in_names),
            out_names=tuple(out_names),
            lowering_input_output_aliases=(),
            sim_require_finite=True,
            sim_require_nnan=True,
            nc=nc,
        )
        return tuple(outs)

    devices = jax.devices()[:N_CORES]
    mesh = Mesh(np.asarray(devices), ("core",))
    in_specs = (PartitionSpec("core"),) * (n_params + n_outs)
    out_specs = (PartitionSpec("core"),) * len(out_names)
    donate_argnums = tuple(range(n_params, n_params + n_outs)) if donate else ()
    fn = jax.jit(
        shard_map(
            _body, mesh=mesh, in_specs=in_specs, out_specs=out_specs, check_rep=False
        ),
        donate_argnums=donate_argnums,
        keep_unused=True,
    )
    sharding = NamedSharding(mesh, PartitionSpec("core"))
    return fn, in_names, out_names, out_avals, zero_shapes, sharding


def _make_runner(nc):
    import jax

    fn, in_names, out_names, out_avals, zero_shapes, sharding = _build_jit(nc)

    def run(in_maps):
        concat_in = [
            np.concatenate(
                [np.asarray(in_maps[c][nm]) for c in range(N_CORES)], axis=0
            )
            for nm in in_names
        ]
        zeros = [np.zeros((N_CORES * s[0], *s[1:]), dt) for s, dt in zero_shapes]
        dev_in = [jax.device_put(x, sharding) for x in concat_in]
        out_arrs = fn(*dev_in, *zeros)
        return [
            {
                nm: np.asarray(out_arrs[i]).reshape(N_CORES, *out_avals[i].shape)[c]
                for i, nm in enumerate(out_names)
            }
            for c in range(N_CORES)
        ]

    return run


def kernel(hidden_states, weight):
    hs = np.asarray(hidden_states, dtype=np.float32)
    w = np.asarray(weight, dtype=np.float32)
    assert hs.shape == (NUM_TOKENS, HIDDEN), hs.shape
    assert w.shape == (NUM_EXPERTS, HIDDEN), w.shape

    in_maps = _prep_inputs(hs, w)
    nc = _get_nc()
    try:
        if "run" not in _FN_CACHE:
            _FN_CACHE["run"] = _make_runner(nc)
        results = _FN_CACHE["run"](in_maps)
    except Exception:
        # fall back to the stock path if the cached-runner path breaks
        from concourse.bass_utils import run_bass_kernel_spmd

        results = run_bass_kernel_spmd(
            nc, in_maps, core_ids=list(range(N_CORES))
        ).results

    topk_idx = np.concatenate([r["topk_idx"] for r in results], axis=0)
    topk_w = np.concatenate([r["topk_w"] for r in results], axis=0)
    row_idx = (
        np.arange(NUM_TOKENS * TOP_K, dtype=np.int32).reshape(TOP_K, NUM_TOKENS).T
    )
    return (
        topk_idx.astype(np.int32),
        topk_w.astype(np.float32),
        row_idx,
    )
